# revision 1
# baseline (speedup 1.0000x reference)
"""SATD loss kernel for Trainium2: sum |H @ (original - pred)|.

Full inputs: original, pred [2, 8192, 64, 64] f32. H is the 64x64
Sylvester Hadamard matrix applied along axis -2 of each 64x64 block.

Strategy (8-way data parallel over the 16384 blocks, 2048 per core):
  - Host: shard blocks across cores, cast to bf16 (H has +-1 entries;
    the transform accumulates in fp32 PSUM, and the bf16 input rounding
    contributes ~1e-6 relative error on the final scalar), and repack
    each core's data into [T, 128, 2*COLS] tiles whose partition axis
    holds the j-rows of 128 blocks (two 64-block halves m=0/1 on
    partitions 0-63 / 64-127) and whose free axis is (g, k) for
    original then pred.
  - Device, per tile: one contiguous 4 MiB DMA; per 512-column slice,
    matmul with lhsT = kron(I2, H) on the original half, then
    accumulate matmul with -kron(I2, H) on the pred half into the same
    PSUM bank -> PSUM = H @ (A - B) for 16 blocks at 128 partitions.
  - Fused abs+sum (tensor_reduce apply_absolute_value on VectorE) per
    PSUM bank into an SBUF accumulator; final reduce -> [128, 1]/core.
  - Host sums the 8x128 partials (f64) and casts to f32.
"""

from contextlib import ExitStack

import ml_dtypes
import numpy as np

import concourse.bass as bass
import concourse.tile as tile
from concourse import bacc, mybir
from concourse.bass_utils import run_bass_kernel_spmd

N_CORES = 8
N = 64                       # Hadamard block size
BLOCKS_TOTAL = 2 * 8192      # 16384 blocks of [64, 64]
BLOCKS_PER_CORE = BLOCKS_TOTAL // N_CORES   # 2048
G = 128                      # blocks per partition-half per tile
COLS = G * N                 # 4096 bf16 = 8 KiB per partition per input
TILES = BLOCKS_PER_CORE // (2 * G)          # 16 iterations
MM_N = 512                   # matmul moving free dim (one PSUM bank)
SUB = COLS // MM_N           # psum tiles per SBUF tile (8)

F32 = mybir.dt.float32
# Input quantization: "bf16" (rel err ~1e-6) or "e4m3" (rel err ~4e-4,
# half the DMA traffic). PSUM accumulation is fp32 either way.
import os
QUANT = os.environ.get("SATD_QUANT", "e4m3")
if QUANT == "e4m3":
    IN_DT = mybir.dt.float8e4
    IN_NP = ml_dtypes.float8_e4m3
else:
    IN_DT = mybir.dt.bfloat16
    IN_NP = ml_dtypes.bfloat16


def _hadamard(n: int) -> np.ndarray:
    H = np.array([[1.0]], dtype=np.float32)
    while H.shape[0] < n:
        H = np.block([[H, H], [H, -H]])
    return H.astype(np.float32)


def _weights() -> np.ndarray:
    # lhsT for out = Hd @ rhs is Hd.T; kron(I2, H) is symmetric.
    Hd = np.kron(np.eye(2, dtype=np.float32), _hadamard(N))
    return np.concatenate([Hd, -Hd], axis=1).astype(
        IN_NP)  # [128, 256], entries +-1 exact in bf16/e4m3


def _build_program() -> bacc.Bacc:
    nc = bacc.Bacc("TRN2", target_bir_lowering=False, debug=False,
                   num_devices=N_CORES)
    x = nc.dram_tensor("x", [TILES, 128, 2 * COLS], IN_DT,
                       kind="ExternalInput").ap()
    w = nc.dram_tensor("w", [128, 256], IN_DT, kind="ExternalInput").ap()
    out = nc.dram_tensor("out", [128, 2], F32, kind="ExternalOutput").ap()

    with tile.TileContext(nc) as tc, ExitStack() as ctx:
        wpool = ctx.enter_context(tc.tile_pool(name="w", bufs=1))
        xpool = ctx.enter_context(tc.tile_pool(name="x", bufs=3))
        psum = ctx.enter_context(tc.tile_pool(name="psum", bufs=4,
                                              space="PSUM"))
        accpool = ctx.enter_context(tc.tile_pool(name="acc", bufs=1))
        scratch = ctx.enter_context(tc.tile_pool(name="scr", bufs=2))

        wt = wpool.tile([128, 256], IN_DT)
        nc.sync.dma_start(wt[:], w[:])
        w_pos = wt[:, 0:128]
        w_neg = wt[:, 128:256]

        # Separate accumulators per reduce engine so VectorE and ScalarE
        # never touch the same tile (no cross-engine serialization).
        npairs = TILES * SUB // 2
        accv = accpool.tile([128, 5 * (npairs // 8)], F32, tag="accv")
        acca = accpool.tile([128, 3 * (npairs // 8)], F32, tag="acca")

        w3 = wt[:].rearrange("p (h m) -> p h m", h=2)

        for t in range(TILES):
            xt = xpool.tile([128, 2 * COLS], IN_DT)
            # Host layout interleaves the original/pred halves per
            # 512-column group: xt cols = (s, h, c). Any contiguous
            # column range is then self-contained, so the first and
            # last tiles stream in chunks (faster pipeline fill/drain)
            # while middle tiles use one large DMA.
            n_chunks = 4 if t in (0, TILES - 1) else 1
            step = SUB // n_chunks
            for c0 in range(0, SUB, step):
                lo, hi = c0 * 2 * MM_N, (c0 + step) * 2 * MM_N
                nc.sync.dma_start(xt[:, lo:hi], x[t, :, lo:hi])
            # DoubleRow contracts over (p, h) in one pass: a single
            # matmul computes Hd@A - Hd@B per 512-column group. Pairs of
            # groups share a 2-bank PSUM tile and one abs+sum; VectorE
            # (lower per-op cost) takes 5 of every 8 pairs, ScalarE 3.
            for pr in range(SUB // 2):
                pt = psum.tile([128, 2 * MM_N], F32)
                for q in range(2):
                    s = pr * 2 + q
                    x3 = xt[:, s * 2 * MM_N:(s + 1) * 2 * MM_N].rearrange(
                        "p (h c) -> p h c", h=2)
                    nc.tensor.matmul(pt[:, q * MM_N:(q + 1) * MM_N], w3, x3,
                                     start=True, stop=True,
                                     perf_mode=mybir.MatmulPerfMode.DoubleRow)
                k = t * (SUB // 2) + pr
                if pr % 8 < 5:
                    col = 5 * (k // 8) + k % 8
                    nc.vector.tensor_reduce(
                        accv[:, col:col + 1], pt[:],
                        axis=mybir.AxisListType.X, op=mybir.AluOpType.add,
                        apply_absolute_value=True)
                else:
                    col = 3 * (k // 8) + k % 8 - 5
                    st = scratch.tile([128, 2 * MM_N], F32)
                    nc.scalar.activation(
                        st[:], pt[:], mybir.ActivationFunctionType.Abs,
                        accum_out=acca[:, col:col + 1])

        res = accpool.tile([128, 2], F32, tag="res")
        nc.vector.tensor_reduce(res[:, 0:1], accv[:],
                                axis=mybir.AxisListType.X,
                                op=mybir.AluOpType.add)
        nc.vector.tensor_reduce(res[:, 1:2], acca[:],
                                axis=mybir.AxisListType.X,
                                op=mybir.AluOpType.add)
        nc.sync.dma_start(out[:], res[:])

    nc.compile()
    return nc


def _repack(shard: np.ndarray) -> np.ndarray:
    """[BLOCKS_PER_CORE, 64, 64] -> [TILES, 128, SUB, COLS // SUB] with
    partition axis (m, j) and free axis (g, k) split into SUB groups of
    512 columns (8 g-blocks each)."""
    v = shard.reshape(TILES, 2, G, N, N)          # t, m, g, j, k
    v = v.transpose(0, 1, 3, 2, 4)                # t, m, j, g, k
    return v.reshape(TILES, 128, SUB, MM_N)


_NC = None


def _get_program() -> bacc.Bacc:
    global _NC
    if _NC is None:
        _NC = _build_program()
    return _NC


def _run(original: np.ndarray, pred: np.ndarray, **spmd_kwargs):
    a_full = np.asarray(original, dtype=np.float32).reshape(
        BLOCKS_TOTAL, N, N).astype(IN_NP)
    b_full = np.asarray(pred, dtype=np.float32).reshape(
        BLOCKS_TOTAL, N, N).astype(IN_NP)
    wnp = _weights()
    in_maps = []
    for i in range(N_CORES):
        sl = slice(i * BLOCKS_PER_CORE, (i + 1) * BLOCKS_PER_CORE)
        xi = np.empty((TILES, 128, SUB, 2, MM_N), dtype=IN_NP)
        xi[:, :, :, 0, :] = _repack(a_full[sl])
        xi[:, :, :, 1, :] = _repack(b_full[sl])
        in_maps.append({"x": xi.reshape(TILES, 128, 2 * COLS), "w": wnp})
    nc = _get_program()
    r = run_bass_kernel_spmd(nc, in_maps, list(range(N_CORES)),
                             **spmd_kwargs)
    total = 0.0
    for i in range(N_CORES):
        total += r.results[i]["out"].astype(np.float64).sum()
    return np.float32(total), r


def kernel(original: np.ndarray, pred: np.ndarray) -> np.ndarray:
    val, _ = _run(original, pred)
    return np.array(val, dtype=np.float32)



# revision 2
# speedup vs baseline: 1.2436x; 1.2436x over previous
"""SATD loss kernel for Trainium2: sum |H @ (original - pred)|.

Full inputs: original, pred [2, 8192, 64, 64] f32. H is the 64x64
Sylvester Hadamard matrix applied along axis -2 of each 64x64 block.

Strategy (8-way data parallel over the 16384 blocks, 2048 per core):
  - Host: d = original - pred (exact, H is linear), cast to fp8 e4m3
    (quantizing the difference contributes ~2e-4 relative error on the
    final scalar; the transform accumulates in fp32 PSUM), and repack
    each core's shard into [T=8, 128, 8192] tiles whose partition axis
    holds the j-rows of 2 block-halves (m=0/1 on partitions 0-63 /
    64-127) and whose free axis is (g, k) for 128 blocks per tile.
  - Device, per tile: 1 MiB DMA; per 512-column group one matmul with
    lhsT = kron(I2, H) (fp8, +-1 exact) -> PSUM = H @ d for 16 blocks.
  - PSUM evacuation is the wall (only VectorE and ScalarE can read
    PSUM, 1 elem/lane/cycle at 0.96 / 1.2 GHz): pairs of groups share
    a 2-bank PSUM tile; fused abs+sum per 2-bank tile, statically
    split ~35:29 between VectorE (tensor_reduce, ~1.17 ns/col) and
    ScalarE (activation Abs + accum, ~1.36 ns/col) so both engines
    finish together. Each engine's tiles come from its own
    double-buffered 2-bank pool (4 tiles x 2 banks = all 8 banks).
  - Final reduce -> [128, 2] per core; host sums partials in f64.
"""

from contextlib import ExitStack

import ml_dtypes
import numpy as np

import concourse.bass as bass
import concourse.tile as tile
from concourse import bacc, mybir
from concourse.bass_utils import run_bass_kernel_spmd

N_CORES = 8
N = 64                       # Hadamard block size
BLOCKS_TOTAL = 2 * 8192      # 16384 blocks of [64, 64]
BLOCKS_PER_CORE = BLOCKS_TOTAL // N_CORES   # 2048
G = 128                      # blocks per tile (2 per partition-column)
COLS = G * N // 2            # 8192 fp8 = 8 KiB per partition per tile
TILES = BLOCKS_PER_CORE // G                # 8
MM_N = 512                   # matmul moving free dim (one PSUM bank)
UNITS = TILES * COLS // 1024                # 64 2-bank reduce units/core

F32 = mybir.dt.float32
IN_DT = mybir.dt.float8e4
IN_NP = ml_dtypes.float8_e4m3

# Static DVE:ACT split of the 64 reduce units. Measured sustained
# rates: DVE 1024 cols / ~1166 ns, ACT 1024 / ~1396 ns -> 35:29.
DVE_UNITS = 35
_IS_DVE = [((u + 1) * DVE_UNITS) // UNITS > (u * DVE_UNITS) // UNITS
           for u in range(UNITS)]


def _hadamard(n: int) -> np.ndarray:
    H = np.array([[1.0]], dtype=np.float32)
    while H.shape[0] < n:
        H = np.block([[H, H], [H, -H]])
    return H.astype(np.float32)


def _weights() -> np.ndarray:
    # lhsT for out = Hd @ rhs is Hd.T; kron(I2, H) is symmetric.
    return np.kron(np.eye(2, dtype=np.float32), _hadamard(N)).astype(
        IN_NP)  # [128, 128], entries +-1 exact in fp8


def _build_program() -> bacc.Bacc:
    nc = bacc.Bacc("TRN2", target_bir_lowering=False, debug=False,
                   num_devices=N_CORES)
    x = nc.dram_tensor("x", [TILES, 128, COLS], IN_DT,
                       kind="ExternalInput").ap()
    w = nc.dram_tensor("w", [128, 128], IN_DT, kind="ExternalInput").ap()
    out = nc.dram_tensor("out", [128, 2], F32, kind="ExternalOutput").ap()

    with tile.TileContext(nc) as tc, ExitStack() as ctx:
        wpool = ctx.enter_context(tc.tile_pool(name="w", bufs=1))
        xpool = ctx.enter_context(tc.tile_pool(name="x", bufs=3))
        vpsum = ctx.enter_context(tc.tile_pool(name="vpsum", bufs=2,
                                               space="PSUM"))
        apsum = ctx.enter_context(tc.tile_pool(name="apsum", bufs=2,
                                               space="PSUM"))
        accpool = ctx.enter_context(tc.tile_pool(name="acc", bufs=1))
        scratch = ctx.enter_context(tc.tile_pool(name="scr", bufs=2))

        wt = wpool.tile([128, 128], IN_DT)
        nc.sync.dma_start(wt[:], w[:])

        n_a = UNITS - DVE_UNITS
        accv = accpool.tile([128, DVE_UNITS], F32, tag="accv")
        acca = accpool.tile([128, n_a], F32, tag="acca")

        vi = ai = 0
        for t in range(TILES):
            xt = xpool.tile([128, COLS], IN_DT)
            # First tile streams in 2048-col chunks so the pipeline
            # fills fast; later tiles use one 1 MiB DMA.
            n_chunks = 4 if t == 0 else 1
            step = COLS // n_chunks
            for c0 in range(0, COLS, step):
                nc.sync.dma_start(xt[:, c0:c0 + step], x[t, :, c0:c0 + step])
            for i in range(COLS // 1024):
                u = t * (COLS // 1024) + i
                pool = vpsum if _IS_DVE[u] else apsum
                pt = pool.tile([128, 1024], F32)
                for q in range(2):
                    lo = i * 1024 + q * MM_N
                    nc.tensor.matmul(pt[:, q * MM_N:(q + 1) * MM_N],
                                     wt[:], xt[:, lo:lo + MM_N],
                                     start=True, stop=True)
                if _IS_DVE[u]:
                    nc.vector.tensor_reduce(
                        accv[:, vi:vi + 1], pt[:],
                        axis=mybir.AxisListType.X, op=mybir.AluOpType.add,
                        apply_absolute_value=True)
                    vi += 1
                else:
                    st = scratch.tile([128, 1024], mybir.dt.bfloat16)
                    nc.scalar.activation(
                        st[:], pt[:], mybir.ActivationFunctionType.Abs,
                        accum_out=acca[:, ai:ai + 1])
                    ai += 1

        res = accpool.tile([128, 2], F32, tag="res")
        nc.vector.tensor_reduce(res[:, 0:1], accv[:],
                                axis=mybir.AxisListType.X,
                                op=mybir.AluOpType.add)
        nc.vector.tensor_reduce(res[:, 1:2], acca[:],
                                axis=mybir.AxisListType.X,
                                op=mybir.AluOpType.add)
        nc.sync.dma_start(out[:], res[:])

    nc.compile()
    return nc


def _repack(shard: np.ndarray) -> np.ndarray:
    """[BLOCKS_PER_CORE, 64, 64] fp8 -> [TILES, 128, COLS] with
    partition axis (m, j) and free axis (g, k)."""
    v = shard.reshape(TILES, 2, G // 2, N, N)     # t, m, g, j, k
    v = v.transpose(0, 1, 3, 2, 4)                # t, m, j, g, k
    return v.reshape(TILES, 128, COLS)


_NC = None


def _get_program() -> bacc.Bacc:
    global _NC
    if _NC is None:
        _NC = _build_program()
    return _NC


def _run(original: np.ndarray, pred: np.ndarray, **spmd_kwargs):
    a = np.asarray(original, dtype=np.float32).reshape(BLOCKS_TOTAL, N, N)
    b = np.asarray(pred, dtype=np.float32).reshape(BLOCKS_TOTAL, N, N)
    d_full = (a - b).astype(IN_NP)
    wnp = _weights()
    in_maps = []
    for i in range(N_CORES):
        sl = slice(i * BLOCKS_PER_CORE, (i + 1) * BLOCKS_PER_CORE)
        in_maps.append({"x": _repack(d_full[sl]), "w": wnp})
    nc = _get_program()
    r = run_bass_kernel_spmd(nc, in_maps, list(range(N_CORES)),
                             **spmd_kwargs)
    total = 0.0
    for i in range(N_CORES):
        total += r.results[i]["out"].astype(np.float64).sum()
    return np.float32(total), r


def kernel(original: np.ndarray, pred: np.ndarray) -> np.ndarray:
    val, _ = _run(original, pred)
    return np.array(val, dtype=np.float32)


# revision 4
# speedup vs baseline: 1.2887x; 1.0363x over previous
"""SATD loss kernel for Trainium2: sum |H @ (original - pred)|.

Full inputs: original, pred [2, 8192, 64, 64] f32. H is the 64x64
Sylvester Hadamard matrix applied along axis -2 of each 64x64 block.

Strategy (8-way data parallel over the 16384 blocks, 2048 per core):
  - Host: d = original - pred (exact, H is linear), cast to fp8 e4m3
    (quantizing the difference contributes ~2e-4 relative error on the
    final scalar; the transform accumulates in fp32 PSUM), and repack
    each core's shard into [T=8, 128, 8192] tiles whose partition axis
    holds the j-rows of 2 block-halves (m=0/1 on partitions 0-63 /
    64-127) and whose free axis is (g, k) for 128 blocks per tile.
  - Device, per tile: 1 MiB DMA; per 512-column group one matmul with
    lhsT = kron(I2, H) (fp8, +-1 exact) -> PSUM = H @ d for 16 blocks.
  - PSUM evacuation is the wall (only VectorE and ScalarE can read
    PSUM, 1 elem/lane/cycle at 0.96 / 1.2 GHz): pairs of groups share
    a 2-bank PSUM tile; fused abs+sum per 2-bank tile, statically
    split ~35:29 between VectorE (tensor_reduce, ~1.17 ns/col) and
    ScalarE (activation Abs + accum, ~1.36 ns/col) so both engines
    finish together. Each engine's tiles come from its own
    double-buffered 2-bank pool (4 tiles x 2 banks = all 8 banks).
  - Final reduce -> [128, 2] per core; host sums partials in f64.
"""

from contextlib import ExitStack

import ml_dtypes
import numpy as np

import concourse.bass as bass
import concourse.tile as tile
from concourse import bacc, mybir
from concourse.bass_utils import run_bass_kernel_spmd

N_CORES = 8
N = 64                       # Hadamard block size
BLOCKS_TOTAL = 2 * 8192      # 16384 blocks of [64, 64]
BLOCKS_PER_CORE = BLOCKS_TOTAL // N_CORES   # 2048
G = 128                      # blocks per tile (2 per partition-column)
COLS = G * N // 2            # 8192 fp8 = 8 KiB per partition per tile
TILES = BLOCKS_PER_CORE // G                # 8
MM_N = 512                   # matmul moving free dim (one PSUM bank)
UNITS = TILES * COLS // 1024                # 64 2-bank reduce units/core

F32 = mybir.dt.float32
IN_DT = mybir.dt.float8e4
IN_NP = ml_dtypes.float8_e4m3

# Static DVE:ACT split of the 64 reduce units. Measured sustained
# rates: DVE 1024 cols / ~1166 ns, ACT 1024 / ~1396 ns -> 35:29.
DVE_UNITS = 35
_IS_DVE = [((u + 1) * DVE_UNITS) // UNITS > (u * DVE_UNITS) // UNITS
           for u in range(UNITS)]


def _hadamard(n: int) -> np.ndarray:
    H = np.array([[1.0]], dtype=np.float32)
    while H.shape[0] < n:
        H = np.block([[H, H], [H, -H]])
    return H.astype(np.float32)


def _weights() -> np.ndarray:
    # lhsT for out = Hd @ rhs is Hd.T; kron(I2, H) is symmetric.
    return np.kron(np.eye(2, dtype=np.float32), _hadamard(N)).astype(
        IN_NP)  # [128, 128], entries +-1 exact in fp8


def _build_program() -> bacc.Bacc:
    nc = bacc.Bacc("TRN2", target_bir_lowering=False, debug=False,
                   num_devices=N_CORES)
    x = nc.dram_tensor("x", [TILES, 128, COLS], IN_DT,
                       kind="ExternalInput").ap()
    w = nc.dram_tensor("w", [128, 128], IN_DT, kind="ExternalInput").ap()
    out = nc.dram_tensor("out", [128, 2], F32, kind="ExternalOutput").ap()

    with tile.TileContext(nc) as tc, ExitStack() as ctx:
        wpool = ctx.enter_context(tc.tile_pool(name="w", bufs=1))
        xpool = ctx.enter_context(tc.tile_pool(name="x", bufs=5))
        vpsum = ctx.enter_context(tc.tile_pool(name="vpsum", bufs=2,
                                               space="PSUM"))
        apsum = ctx.enter_context(tc.tile_pool(name="apsum", bufs=2,
                                               space="PSUM"))
        accpool = ctx.enter_context(tc.tile_pool(name="acc", bufs=1))
        scratch = ctx.enter_context(tc.tile_pool(name="scr", bufs=2))

        wt = wpool.tile([128, 128], IN_DT)
        nc.sync.dma_start(wt[:], w[:])

        n_a = UNITS - DVE_UNITS
        accv = accpool.tile([128, DVE_UNITS], F32, tag="accv")
        acca = accpool.tile([128, n_a], F32, tag="acca")

        vi = ai = 0
        for t in range(TILES):
            xt = xpool.tile([128, COLS], IN_DT)
            # First/last tiles stream in 2048-col chunks (faster
            # pipeline fill and drain); middle tiles use one 1 MiB DMA.
            n_chunks = 4 if t in (0, TILES - 1) else 1
            step = COLS // n_chunks
            for c0 in range(0, COLS, step):
                nc.sync.dma_start(xt[:, c0:c0 + step], x[t, :, c0:c0 + step])
            for i in range(COLS // 1024):
                u = t * (COLS // 1024) + i
                pool = vpsum if _IS_DVE[u] else apsum
                pt = pool.tile([128, 1024], F32)
                for q in range(2):
                    lo = i * 1024 + q * MM_N
                    nc.tensor.matmul(pt[:, q * MM_N:(q + 1) * MM_N],
                                     wt[:], xt[:, lo:lo + MM_N],
                                     start=True, stop=True)
                if _IS_DVE[u]:
                    nc.vector.tensor_reduce(
                        accv[:, vi:vi + 1], pt[:],
                        axis=mybir.AxisListType.X, op=mybir.AluOpType.add,
                        apply_absolute_value=True)
                    vi += 1
                else:
                    st = scratch.tile([128, 1024], mybir.dt.bfloat16)
                    nc.scalar.activation(
                        st[:], pt[:], mybir.ActivationFunctionType.Abs,
                        accum_out=acca[:, ai:ai + 1])
                    ai += 1

        res = accpool.tile([128, 2], F32, tag="res")
        nc.vector.tensor_reduce(res[:, 0:1], accv[:],
                                axis=mybir.AxisListType.X,
                                op=mybir.AluOpType.add)
        nc.vector.tensor_reduce(res[:, 1:2], acca[:],
                                axis=mybir.AxisListType.X,
                                op=mybir.AluOpType.add)
        nc.sync.dma_start(out[:], res[:])

    nc.compile()
    return nc


def _repack(shard: np.ndarray) -> np.ndarray:
    """[BLOCKS_PER_CORE, 64, 64] fp8 -> [TILES, 128, COLS] with
    partition axis (m, j) and free axis (g, k)."""
    v = shard.reshape(TILES, 2, G // 2, N, N)     # t, m, g, j, k
    v = v.transpose(0, 1, 3, 2, 4)                # t, m, j, g, k
    return v.reshape(TILES, 128, COLS)


_NC = None


def _get_program() -> bacc.Bacc:
    global _NC
    if _NC is None:
        _NC = _build_program()
    return _NC


def _run(original: np.ndarray, pred: np.ndarray, **spmd_kwargs):
    a = np.asarray(original, dtype=np.float32).reshape(BLOCKS_TOTAL, N, N)
    b = np.asarray(pred, dtype=np.float32).reshape(BLOCKS_TOTAL, N, N)
    d_full = (a - b).astype(IN_NP)
    wnp = _weights()
    in_maps = []
    for i in range(N_CORES):
        sl = slice(i * BLOCKS_PER_CORE, (i + 1) * BLOCKS_PER_CORE)
        in_maps.append({"x": _repack(d_full[sl]), "w": wnp})
    nc = _get_program()
    r = run_bass_kernel_spmd(nc, in_maps, list(range(N_CORES)),
                             **spmd_kwargs)
    total = 0.0
    for i in range(N_CORES):
        total += r.results[i]["out"].astype(np.float64).sum()
    return np.float32(total), r


def kernel(original: np.ndarray, pred: np.ndarray) -> np.ndarray:
    val, _ = _run(original, pred)
    return np.array(val, dtype=np.float32)


# revision 13
# speedup vs baseline: 1.3390x; 1.0390x over previous
"""SATD loss kernel for Trainium2: sum |H @ (original - pred)|.

Full inputs: original, pred [2, 8192, 64, 64] f32. H is the 64x64
Sylvester Hadamard matrix applied along axis -2 of each 64x64 block.

Strategy (8-way data parallel over the 16384 blocks, 2048 per core):
  - Host: d = original - pred (exact, H is linear), cast to fp8 e4m3
    (quantizing the difference contributes ~2e-4 relative error on the
    final scalar; the transform accumulates in fp32 PSUM), and repack
    each core's shard into [T=8, 128, 8192] tiles whose partition axis
    holds the j-rows of 2 block-halves (m=0/1 on partitions 0-63 /
    64-127) and whose free axis is (g, k) for 128 blocks per tile.
  - Device, per tile: 1 MiB DMA; per 512-column group one matmul with
    lhsT = kron(I2, H) (fp8, +-1 exact) -> PSUM = H @ d for 16 blocks.
  - PSUM evacuation is the wall (only VectorE and ScalarE can read
    PSUM, 1 elem/lane/cycle at 0.96 / 1.2 GHz): pairs of groups share
    a 2-bank PSUM tile; fused abs+sum per 2-bank tile, statically
    split ~35:29 between VectorE (tensor_reduce, ~1.17 ns/col) and
    ScalarE (activation Abs + accum, ~1.36 ns/col) so both engines
    finish together. Each engine's tiles come from its own
    double-buffered 2-bank pool (4 tiles x 2 banks = all 8 banks).
  - Final reduce -> [128, 2] per core; host sums partials in f64.
"""

from contextlib import ExitStack

import ml_dtypes
import numpy as np

import concourse.bass as bass
import concourse.tile as tile
from concourse import bacc, mybir
from concourse.bass_utils import run_bass_kernel_spmd

N_CORES = 8
N = 64                       # Hadamard block size
BLOCKS_TOTAL = 2 * 8192      # 16384 blocks of [64, 64]
BLOCKS_PER_CORE = BLOCKS_TOTAL // N_CORES   # 2048
G = 128                      # blocks per tile (2 per partition-column)
COLS = G * N // 2            # 8192 fp8 = 8 KiB per partition per tile
TILES = BLOCKS_PER_CORE // G                # 8
MM_N = 512                   # matmul moving free dim (one PSUM bank)
UNITS = TILES * COLS // 1024                # 64 2-bank reduce units/core

F32 = mybir.dt.float32
IN_DT = mybir.dt.float8e4
IN_NP = ml_dtypes.float8_e4m3

# Static DVE:ACT split of the 64 reduce units. Measured sustained
# rates: DVE 1024 cols / ~1166 ns, ACT 1024 / ~1396 ns -> 35:29.
DVE_UNITS = 35
_IS_DVE = [((u + 1) * DVE_UNITS) // UNITS > (u * DVE_UNITS) // UNITS
           for u in range(UNITS)]


def _hadamard(n: int) -> np.ndarray:
    H = np.array([[1.0]], dtype=np.float32)
    while H.shape[0] < n:
        H = np.block([[H, H], [H, -H]])
    return H.astype(np.float32)


def _weights() -> np.ndarray:
    # lhsT for out = Hd @ rhs is Hd.T; kron(I2, H) is symmetric.
    return np.kron(np.eye(2, dtype=np.float32), _hadamard(N)).astype(
        IN_NP)  # [128, 128], entries +-1 exact in fp8


def _build_program() -> bacc.Bacc:
    nc = bacc.Bacc("TRN2", target_bir_lowering=False, debug=False,
                   num_devices=N_CORES)
    x = nc.dram_tensor("x", [TILES, 128, COLS], IN_DT,
                       kind="ExternalInput").ap()
    w = nc.dram_tensor("w", [128, 128], IN_DT, kind="ExternalInput").ap()
    out = nc.dram_tensor("out", [128, UNITS], F32,
                         kind="ExternalOutput").ap()

    with tile.TileContext(nc) as tc, ExitStack() as ctx:
        wpool = ctx.enter_context(tc.tile_pool(name="w", bufs=1))
        xpool = ctx.enter_context(tc.tile_pool(name="x", bufs=5))
        vpsum = ctx.enter_context(tc.tile_pool(name="vpsum", bufs=2,
                                               space="PSUM"))
        apsum = ctx.enter_context(tc.tile_pool(name="apsum", bufs=2,
                                               space="PSUM"))
        accpool = ctx.enter_context(tc.tile_pool(name="acc", bufs=1))
        scratch = ctx.enter_context(tc.tile_pool(name="scr", bufs=2))

        wt = wpool.tile([128, 128], IN_DT)
        nc.sync.dma_start(wt[:], w[:])

        # One [128, UNITS] accumulator; DVE writes cols 0:DVE_UNITS,
        # ScalarE the rest. Host sums the partials, so no on-device
        # final reduce (shorter drain).
        acc = accpool.tile([128, UNITS], F32, tag="acc")

        # Warm-up burst: ~30 back-to-back matmuls on the weight tile
        # while the first input DMA is in flight. This trips the PE
        # HAM clock gate to K=8/8 (~3.4 us of sustained activity) so
        # the real matmuls run at 2.4 GHz instead of 1.2. The dummies
        # write into unit 0's PSUM tile (overwritten by its real
        # matmuls, which start=True-clear the bank).
        pt_first = (vpsum if _IS_DVE[0] else apsum).tile(
            [128, 1024], F32, tag="pt")
        for _ in range(30):
            nc.tensor.matmul(pt_first[:, 0:128], wt[:], wt[:],
                             start=True, stop=True)

        vi = ai = 0
        for t in range(TILES):
            xt = xpool.tile([128, COLS], IN_DT)
            # First/last tiles stream in 2048-col chunks (faster
            # pipeline fill and drain); middle tiles use one 1 MiB DMA.
            n_chunks = 4 if t in (0, TILES - 1) else 1
            step = COLS // n_chunks
            for c0 in range(0, COLS, step):
                nc.sync.dma_start(xt[:, c0:c0 + step], x[t, :, c0:c0 + step])
            for i in range(COLS // 1024):
                u = t * (COLS // 1024) + i
                if u == 0:
                    pt = pt_first
                else:
                    pool = vpsum if _IS_DVE[u] else apsum
                    pt = pool.tile([128, 1024], F32, tag="pt")
                for q in range(2):
                    lo = i * 1024 + q * MM_N
                    nc.tensor.matmul(pt[:, q * MM_N:(q + 1) * MM_N],
                                     wt[:], xt[:, lo:lo + MM_N],
                                     start=True, stop=True)
                if _IS_DVE[u]:
                    nc.vector.tensor_reduce(
                        acc[:, vi:vi + 1], pt[:],
                        axis=mybir.AxisListType.X, op=mybir.AluOpType.add,
                        apply_absolute_value=True)
                    vi += 1
                else:
                    st = scratch.tile([128, 1024], mybir.dt.bfloat16)
                    c = DVE_UNITS + ai
                    nc.scalar.activation(
                        st[:], pt[:], mybir.ActivationFunctionType.Abs,
                        accum_out=acc[:, c:c + 1])
                    ai += 1

        nc.sync.dma_start(out[:], acc[:])

    nc.compile()
    return nc


def _repack(shard: np.ndarray) -> np.ndarray:
    """[BLOCKS_PER_CORE, 64, 64] fp8 -> [TILES, 128, COLS] with
    partition axis (m, j) and free axis (g, k)."""
    v = shard.reshape(TILES, 2, G // 2, N, N)     # t, m, g, j, k
    v = v.transpose(0, 1, 3, 2, 4)                # t, m, j, g, k
    return v.reshape(TILES, 128, COLS)


_NC = None


def _get_program() -> bacc.Bacc:
    global _NC
    if _NC is None:
        _NC = _build_program()
    return _NC


def _run(original: np.ndarray, pred: np.ndarray, **spmd_kwargs):
    a = np.asarray(original, dtype=np.float32).reshape(BLOCKS_TOTAL, N, N)
    b = np.asarray(pred, dtype=np.float32).reshape(BLOCKS_TOTAL, N, N)
    d_full = (a - b).astype(IN_NP)
    wnp = _weights()
    in_maps = []
    for i in range(N_CORES):
        sl = slice(i * BLOCKS_PER_CORE, (i + 1) * BLOCKS_PER_CORE)
        in_maps.append({"x": _repack(d_full[sl]), "w": wnp})
    nc = _get_program()
    r = run_bass_kernel_spmd(nc, in_maps, list(range(N_CORES)),
                             **spmd_kwargs)
    total = 0.0
    for i in range(N_CORES):
        total += r.results[i]["out"].astype(np.float64).sum()
    return np.float32(total), r


def kernel(original: np.ndarray, pred: np.ndarray) -> np.ndarray:
    val, _ = _run(original, pred)
    return np.array(val, dtype=np.float32)
